# revision 1
# baseline (speedup 1.0000x reference)
"""Conv2DMod (StyleGAN2-style modulated conv) on 8 Trainium2 NeuronCores.

Math (see reference):
    xm   = x * (1 + style)                           # per-sample, per-Cin
    d    = sqrt(||K_f||^2 * H*W + ||s_b||^2 + eps)   # [B,F]
    y    = conv2d_symmetric_pad(xm, K) / d[b,f]

Everything except the conv itself is a per-sample rescale along either
Cin (contraction dim) or F (output dim), and the symmetric padding is
pixel replication (channel-independent). So the whole op folds into a
plain per-sample conv with host-folded weights (0.003% of the FLOPs):
    W_b[ky,kx,cin,f] = K[ky,kx,cin,f] * (1 + s_b[cin]) / d[b,f]

Device strategy (per core, 2 imgs, batch-parallel across cores):
  - x shipped pre-transposed channel-major [img, row, cin128, cinhalf,
    Wpad=130] with symmetric W-padding baked in (H clamping in-loop).
  - Weights stationary: per output block of 4 rows (512 px), accumulate
    36 fp32r matmuls (2 cinhalf x 9 taps x 2 Fhalf) into two PSUM banks
    [128 F, 512 px]:  psum += W_tile[cin,128F].T-less... = lhsT.T @ rhs
    with lhsT = W tile [cin, 128 F], rhs = x window [cin, 4 rows, 128].
    N=512 moving keeps the ~190ns fp32r LDWEIGHTS fully hidden under
    the 213ns stream (measured 119ns/MM at N=256 vs 106.7 ideal).
  - fp32r = FP22 multiply / fp32 accumulate at full PE rate (~1.5e-4).
  - Output stays channel-major [img, Fhalf, 128, H, W] on device
    (contiguous stores); the NHWC transpose happens on the host.
"""
import numpy as np
import orjson

import concourse.bass as bass
import concourse.mybir as mybir
from concourse import tile
from concourse.bass_utils import run_bass_kernel_spmd

F32R = mybir.dt.float32r
F32 = mybir.dt.float32

B, H, W, CIN, F, KH, KW = 16, 128, 128, 256, 256, 3, 3
NCORES = 8
BL = B // NCORES  # imgs per core
WP = W + 2  # symmetric-padded width
NCH = CIN // 128  # cin partition tiles
NFH = F // 128  # F partition tiles
RB = 4  # output rows per block (4*128 = 512 = fp32 moving-dim max)
NBLK = H // RB
EPS = 1e-8

# ---------------------------------------------------------------------------
# BIR wait-count legalizer: the walrus build here supports fewer sync-wait
# commands per instruction than Tile emits (self-loading fp32r Matmult: 1;
# kernel-tail Drain: one per used proc). Hoist excess waits onto NoOps
# injected just before the offender on the same engine queue (queues run
# in order, so gating is preserved).
# ---------------------------------------------------------------------------
_WAIT_LIMIT = 1


def _legalize_waits(bir: dict, limit: int = _WAIT_LIMIT) -> dict:
    ctr = 0
    for fn in bir.get("functions", []):
        for blk in fn.get("blocks", []):
            new_insts = []
            changed = False
            for ins in blk.get("instructions", []):
                si = ins.get("sync_info")
                if si:
                    waits = si.get("on_wait") or []
                    if len(waits) > limit:
                        excess, keep = waits[:-limit], waits[-limit:]
                        for i in range(0, len(excess), limit):
                            new_insts.append(
                                {
                                    "debug": ins.get("debug", 0),
                                    "engine": ins["engine"],
                                    "ins": [],
                                    "name": f"I-wfix{ctr}-{ins['name']}",
                                    "opcode": "NoOp",
                                    "outs": [],
                                    "sync_info": {
                                        "on_update": [],
                                        "on_wait": excess[i : i + limit],
                                    },
                                }
                            )
                            ctr += 1
                        si["on_wait"] = keep
                        changed = True
                new_insts.append(ins)
            if changed:
                blk["instructions"] = new_insts
    return bir


class _LegalBass(bass.Bass):
    def to_json_bytes(self):
        return orjson.dumps(_legalize_waits(orjson.loads(super().to_json_bytes())))


# ---------------------------------------------------------------------------
# Device kernel build
# ---------------------------------------------------------------------------
_NC_CACHE = {}


def _build_nc():
    if "nc" in _NC_CACHE:
        return _NC_CACHE["nc"]
    nc = _LegalBass()
    # Layouts put the SBUF partition dim right before the free dims so every
    # DMA is a straight linear copy.
    # xt[img, row, cin128(part), ch, wpad]
    xt = nc.dram_tensor("xt", [BL, H, 128, NCH, WP], F32R, kind="ExternalInput")
    # wb[img, ch, cin128(part), ky, kx, fh, f128]
    wb = nc.dram_tensor("wb", [BL, NCH, 128, KH, KW, NFH, 128], F32R, kind="ExternalInput")
    # y2[img, fh, f128(part), row, col] — channel-major; host transposes to NHWC
    y2 = nc.dram_tensor("y2", [BL, NFH, 128, H, W], F32, kind="ExternalOutput")

    with tile.TileContext(nc) as tc:
        with (
            tc.tile_pool(name="wpool", bufs=1) as wpool,
            tc.tile_pool(name="rows", bufs=6) as rows,
            tc.tile_pool(name="outs", bufs=6) as outs,
            tc.tile_pool(name="psum", bufs=4, space="PSUM") as psum,
        ):
            # Folded per-sample weights: one tile per (img, cinhalf) holding
            # all 9 taps x 2 F-halves: [128 cin, ky, kx, fh, 128 f].
            # Issued lazily (inside the img loop, after the first row DMA) so
            # the first block's rows aren't queued behind 4.5 MB of weights.
            wt = {}

            # Warm the PE clock (HAM un-throttles after ~3.4us of activity)
            # with scratch matmuls that run during the initial DMA wait, so
            # the first real matmuls issue at 2.4 GHz instead of 1.2 GHz.
            wu = wpool.tile([128, RB * W], F32, tag="warm")
            nc.gpsimd.memset(wu[:], 0.0)
            wup = psum.tile([128, RB * W], F32, tag="acc0")
            for i in range(5):
                nc.tensor.matmul(
                    wup[:], wu[:, 0:128], wu[:], start=(i == 0), stop=(i == 4)
                )

            for img in range(BL):
                for blk in range(NBLK):
                    r0 = blk * RB
                    # input rows r0-1 .. r0+4 (clamped) into one tile
                    rt = rows.tile([128, RB + 2, NCH, WP], F32R)

                    def ld(dst, a, b, img=img, rt=rt):
                        nc.sync.dma_start(
                            rt[:, dst : dst + (b - a)],
                            xt[img, a:b].rearrange("r p c w -> p r c w"),
                        )

                    if blk == 0:
                        ld(0, 0, 1)
                        ld(1, 0, RB + 1)
                    elif blk == NBLK - 1:
                        ld(0, r0 - 1, r0 + RB)
                        ld(RB + 1, H - 1, H)
                    else:
                        ld(0, r0 - 1, r0 + RB + 1)

                    if blk == 0:
                        # split per-ky so the first taps' weights land early
                        for ch in range(NCH):
                            t = wpool.tile(
                                [128, KH, KW, NFH, 128], F32R, tag=f"w{img}{ch}"
                            )
                            for ky in range(KH):
                                nc.sync.dma_start(
                                    t[:, ky : ky + 1], wb[img, ch, :, ky : ky + 1]
                                )
                            wt[img, ch] = t

                    acc0 = psum.tile([128, RB, W], F32, tag="acc0")
                    acc1 = psum.tile([128, RB, W], F32, tag="acc1")
                    accs = [acc0, acc1]
                    k = 0
                    last = KH * KW * NCH - 1
                    for ch in range(NCH):
                        for dy in range(KH):
                            for dx in range(KW):
                                for fh in range(NFH):
                                    nc.tensor.matmul(
                                        accs[fh][:],
                                        wt[img, ch][:, dy, dx, fh, :],
                                        rt[:, dy : dy + RB, ch, dx : dx + W],
                                        start=(k == 0),
                                        stop=(k == last),
                                    )
                                k += 1
                    for fh in range(NFH):
                        ot = outs.tile([128, RB, W], F32)
                        nc.vector.tensor_copy(ot[:], accs[fh][:])
                        nc.sync.dma_start(y2[img, fh, :, r0 : r0 + RB], ot[:])
    _NC_CACHE["nc"] = nc
    return nc


# ---------------------------------------------------------------------------
# Host wrapper
# ---------------------------------------------------------------------------
def _prepare(x, style, kernel):
    x = np.asarray(x, dtype=np.float32)
    style = np.asarray(style, dtype=np.float32)
    kernel = np.asarray(kernel, dtype=np.float32)

    s = style.reshape(B, CIN)
    w_sq = np.sum(np.square(kernel), axis=(0, 1, 2))  # [F]
    s_sq = np.sum(np.square(s), axis=1)  # [B]
    d = np.sqrt(w_sq[None, :] * np.float32(H * W) + s_sq[:, None] + np.float32(EPS))
    # folded per-sample weights [B, kh, kw, Cin, F]
    wbf = kernel[None] * (1.0 + s)[:, None, None, :, None] / d[:, None, None, None, :]
    # -> [B, NCH, 128, kh, kw, NFH, 128]
    wbf = np.ascontiguousarray(
        wbf.reshape(B, KH, KW, NCH, 128, NFH, 128).transpose(0, 3, 4, 1, 2, 5, 6),
        dtype=np.float32,
    )

    xp = np.pad(x, ((0, 0), (0, 0), (1, 1), (0, 0)), mode="symmetric")  # [B,H,WP,CIN]
    # -> [B, H, 128, NCH, WP]
    xt = np.ascontiguousarray(
        xp.transpose(0, 1, 3, 2).reshape(B, H, NCH, 128, WP).transpose(0, 1, 3, 2, 4),
        dtype=np.float32,
    )
    return xt, wbf


def kernel(x, style, kernel, _trace=False, _tmpdir=None):
    xt, wbf = _prepare(x, style, kernel)
    nc = _build_nc()
    in_maps = [
        {"xt": xt[c * BL : (c + 1) * BL], "wb": wbf[c * BL : (c + 1) * BL]}
        for c in range(NCORES)
    ]
    res = run_bass_kernel_spmd(
        nc,
        in_maps,
        core_ids=list(range(NCORES)),
        trace=_trace,
        tmpdir=_tmpdir,
    )
    # [B, NFH, 128, H, W] -> [B, H, W, NFH*128]
    y2 = np.concatenate([res.results[c]["y2"] for c in range(NCORES)], axis=0)
    y = np.ascontiguousarray(
        y2.reshape(B, F, H, W).transpose(0, 2, 3, 1), dtype=np.float32
    )
    LAST_RUN.clear()
    LAST_RUN.update({"exec_time_ns": res.exec_time_ns, "results": res})
    return y


LAST_RUN = {}



# revision 2
# speedup vs baseline: 1.0769x; 1.0769x over previous
"""Conv2DMod (StyleGAN2-style modulated conv) on 8 Trainium2 NeuronCores.

Math (see reference):
    xm   = x * (1 + style)                           # per-sample, per-Cin
    d    = sqrt(||K_f||^2 * H*W + ||s_b||^2 + eps)   # [B,F]
    y    = conv2d_symmetric_pad(xm, K) / d[b,f]

Everything except the conv itself is a per-sample rescale along either
Cin (contraction dim) or F (output dim), and the symmetric padding is
pixel replication (channel-independent). So the whole op folds into a
plain per-sample conv with host-folded weights (0.003% of the FLOPs):
    W_b[ky,kx,cin,f] = K[ky,kx,cin,f] * (1 + s_b[cin]) / d[b,f]

Device strategy (per core, 2 imgs, batch-parallel across cores):
  - x shipped pre-transposed channel-major [img, row, cin128, cinhalf,
    Wpad=130] in bf16 with symmetric W-padding baked in (H clamping
    in-loop).
  - Weights stationary: per output block of 4 rows (512 px), accumulate
    36 bf16 matmuls (2 cinhalf x 9 taps x 2 Fhalf) into two PSUM banks
    [128 F, 512 px] fp32. bf16 keeps the PE at full rate and (unlike
    fp32r, LDWEIGHTS ~188ns) enables FWL so the 128-col weight load
    (~53-107ns) hides completely under the 213ns moving stream.
  - Output stays channel-major [img, fh, 128, H, W] bf16 on device
    (contiguous stores, half the HBM write traffic); the NHWC transpose
    and fp32 cast happen on the host. bf16 quantization gives ~2.4e-3
    rel err vs the 2e-2 budget.
"""
import numpy as np
import orjson
import ml_dtypes

import concourse.bass as bass
import concourse.mybir as mybir
from concourse import tile
from concourse.bass_utils import run_bass_kernel_spmd

BF16 = mybir.dt.bfloat16
F32 = mybir.dt.float32

B, H, W, CIN, F, KH, KW = 16, 128, 128, 256, 256, 3, 3
NCORES = 8
BL = B // NCORES  # imgs per core
WP = W + 2  # symmetric-padded width
NCH = CIN // 128  # cin partition tiles
NFH = F // 128  # F partition tiles
RB = 4  # output rows per block (4*128 = 512 = fp32 PSUM bank)
NBLK = H // RB
EPS = 1e-8

# ---------------------------------------------------------------------------
# BIR wait-count legalizer: the walrus build here supports fewer sync-wait
# commands per instruction than Tile emits (self-loading Matmult: 1;
# kernel-tail Drain: one per used proc). Hoist excess waits onto NoOps
# injected just before the offender on the same engine queue (queues run
# in order, so gating is preserved).
# ---------------------------------------------------------------------------
_WAIT_LIMIT = 1


def _legalize_waits(bir: dict, limit: int = _WAIT_LIMIT) -> dict:
    ctr = 0
    for fn in bir.get("functions", []):
        for blk in fn.get("blocks", []):
            new_insts = []
            changed = False
            for ins in blk.get("instructions", []):
                si = ins.get("sync_info")
                if si:
                    waits = si.get("on_wait") or []
                    if len(waits) > limit:
                        excess, keep = waits[:-limit], waits[-limit:]
                        for i in range(0, len(excess), limit):
                            new_insts.append(
                                {
                                    "debug": ins.get("debug", 0),
                                    "engine": ins["engine"],
                                    "ins": [],
                                    "name": f"I-wfix{ctr}-{ins['name']}",
                                    "opcode": "NoOp",
                                    "outs": [],
                                    "sync_info": {
                                        "on_update": [],
                                        "on_wait": excess[i : i + limit],
                                    },
                                }
                            )
                            ctr += 1
                        si["on_wait"] = keep
                        changed = True
                new_insts.append(ins)
            if changed:
                blk["instructions"] = new_insts
    return bir


class _LegalBass(bass.Bass):
    def to_json_bytes(self):
        return orjson.dumps(_legalize_waits(orjson.loads(super().to_json_bytes())))


# ---------------------------------------------------------------------------
# Device kernel build
# ---------------------------------------------------------------------------
_NC_CACHE = {}


def _build_nc():
    if "nc" in _NC_CACHE:
        return _NC_CACHE["nc"]
    nc = _LegalBass()
    # Layouts put the SBUF partition dim right before the free dims so every
    # DMA is a straight linear copy.
    # xt[img, row, cin128(part), ch, wpad]
    xt = nc.dram_tensor("xt", [BL, H, 128, NCH, WP], BF16, kind="ExternalInput")
    # wb[img, ch, cin128(part), ky, kx, fh, f128]
    wb = nc.dram_tensor("wb", [BL, NCH, 128, KH, KW, NFH, 128], BF16, kind="ExternalInput")
    # y2[img, fh, f128(part), row, col] — channel-major; host transposes to NHWC
    y2 = nc.dram_tensor("y2", [BL, NFH, 128, H, W], BF16, kind="ExternalOutput")

    with tile.TileContext(nc) as tc:
        with (
            tc.tile_pool(name="wpool", bufs=1) as wpool,
            tc.tile_pool(name="rows", bufs=6) as rows,
            tc.tile_pool(name="outs", bufs=6) as outs,
            tc.tile_pool(name="psum", bufs=4, space="PSUM") as psum,
        ):
            # Folded per-sample weights: one tile per (img, cinhalf) holding
            # all 9 taps x 2 F-halves: [128 cin, ky, kx, fh, 128 f].
            # Issued lazily (inside the img loop, after the first row DMA) so
            # the first block's rows aren't queued behind the weights.
            wt = {}

            # Warm the PE clock (HAM un-throttles after ~3.4us of activity)
            # with scratch matmuls that run during the initial DMA wait, so
            # the first real matmuls issue at 2.4 GHz instead of 1.2 GHz.
            wu = wpool.tile([128, RB * W], BF16, tag="warm")
            nc.gpsimd.memset(wu[:], 0.0)
            wup = psum.tile([128, RB * W], F32, tag="acc0")
            for i in range(5):
                nc.tensor.matmul(
                    wup[:], wu[:, 0:128], wu[:], start=(i == 0), stop=(i == 4)
                )

            for img in range(BL):
                for blk in range(NBLK):
                    r0 = blk * RB
                    # input rows r0-1 .. r0+4 (clamped) into one tile
                    rt = rows.tile([128, RB + 2, NCH, WP], BF16)

                    def ld(dst, a, b, img=img, rt=rt):
                        nc.sync.dma_start(
                            rt[:, dst : dst + (b - a)],
                            xt[img, a:b].rearrange("r p c w -> p r c w"),
                        )

                    if blk == 0:
                        ld(0, 0, 1)
                        ld(1, 0, RB + 1)
                    elif blk == NBLK - 1:
                        ld(0, r0 - 1, r0 + RB)
                        ld(RB + 1, H - 1, H)
                    else:
                        ld(0, r0 - 1, r0 + RB + 1)

                    if blk == 0:
                        # split per-ky so the first taps' weights land early
                        for ch in range(NCH):
                            t = wpool.tile(
                                [128, KH, KW, NFH, 128], BF16, tag=f"w{img}{ch}"
                            )
                            for ky in range(KH):
                                nc.sync.dma_start(
                                    t[:, ky : ky + 1], wb[img, ch, :, ky : ky + 1]
                                )
                            wt[img, ch] = t

                    acc0 = psum.tile([128, RB, W], F32, tag="acc0")
                    acc1 = psum.tile([128, RB, W], F32, tag="acc1")
                    accs = [acc0, acc1]
                    k = 0
                    last = KH * KW * NCH - 1
                    for ch in range(NCH):
                        for dy in range(KH):
                            for dx in range(KW):
                                for fh in range(NFH):
                                    nc.tensor.matmul(
                                        accs[fh][:],
                                        wt[img, ch][:, dy, dx, fh, :],
                                        rt[:, dy : dy + RB, ch, dx : dx + W],
                                        start=(k == 0),
                                        stop=(k == last),
                                    )
                                k += 1
                    for fh in range(NFH):
                        ot = outs.tile([128, RB, W], BF16)
                        nc.vector.tensor_copy(ot[:], accs[fh][:])
                        nc.sync.dma_start(y2[img, fh, :, r0 : r0 + RB], ot[:])
    _NC_CACHE["nc"] = nc
    return nc


# ---------------------------------------------------------------------------
# Host wrapper
# ---------------------------------------------------------------------------
def _prepare(x, style, kernel):
    x = np.asarray(x, dtype=np.float32)
    style = np.asarray(style, dtype=np.float32)
    kernel = np.asarray(kernel, dtype=np.float32)

    s = style.reshape(B, CIN)
    w_sq = np.sum(np.square(kernel), axis=(0, 1, 2))  # [F]
    s_sq = np.sum(np.square(s), axis=1)  # [B]
    d = np.sqrt(w_sq[None, :] * np.float32(H * W) + s_sq[:, None] + np.float32(EPS))
    # folded per-sample weights [B, kh, kw, Cin, F]
    wbf = kernel[None] * (1.0 + s)[:, None, None, :, None] / d[:, None, None, None, :]
    # -> [B, NCH, 128, kh, kw, NFH, 128]
    wbf = np.ascontiguousarray(
        wbf.reshape(B, KH, KW, NCH, 128, NFH, 128).transpose(0, 3, 4, 1, 2, 5, 6)
    ).astype(ml_dtypes.bfloat16)

    xp = np.pad(x, ((0, 0), (0, 0), (1, 1), (0, 0)), mode="symmetric")  # [B,H,WP,CIN]
    # -> [B, H, 128, NCH, WP]
    xt = np.ascontiguousarray(
        xp.transpose(0, 1, 3, 2).reshape(B, H, NCH, 128, WP).transpose(0, 1, 3, 2, 4)
    ).astype(ml_dtypes.bfloat16)
    return xt, wbf


def kernel(x, style, kernel, _trace=False, _tmpdir=None):
    xt, wbf = _prepare(x, style, kernel)
    nc = _build_nc()
    in_maps = [
        {"xt": xt[c * BL : (c + 1) * BL], "wb": wbf[c * BL : (c + 1) * BL]}
        for c in range(NCORES)
    ]
    res = run_bass_kernel_spmd(
        nc,
        in_maps,
        core_ids=list(range(NCORES)),
        trace=_trace,
        tmpdir=_tmpdir,
    )
    # [B, NFH, 128, H, W] -> [B, H, W, NFH*128]
    y2 = np.concatenate(
        [np.asarray(res.results[c]["y2"]) for c in range(NCORES)], axis=0
    ).astype(np.float32)
    y = np.ascontiguousarray(
        y2.reshape(B, F, H, W).transpose(0, 2, 3, 1), dtype=np.float32
    )
    LAST_RUN.clear()
    LAST_RUN.update({"exec_time_ns": res.exec_time_ns, "results": res})
    return y


LAST_RUN = {}
